# revision 23
# baseline (speedup 1.0000x reference)
"""MultiHeadAttention kernel for 8 trn2 NeuronCores (Bass/Tile).

Problem: B=2, S=2048, E=1024, H=16, D=64 (fp32), boolean mask [B,S,S].
  out = softmax(mask((q W_q^T) (k W_k^T)^T / sqrt(D))) (v W_v^T) W_o^T + b_o

Sharding: batch x head-group. Core c (c = 4*g + r) handles batch g and heads
4r..4r+3. Per core:
  - QKV projections for its 4 heads (fp16 matmuls, fp32 PSUM accumulate);
    host pre-casts x and W to fp16 so no on-device conversion is needed
  - attention in transposed layout (scores.T = [k_tok, q_tok]), processing
    the two head-pairs sequentially per q-block:
      PE QK (2 matmuls per k-chunk), ACT exp straight out of PSUM, mask
      multiply alternating DVE/Pool (fp16 2x mode), then ONE AV matmul per
      head whose stationary is [v_h | ones64]: rows 0-63 of the PSUM tile
      accumulate attn*V, rows 64-127 accumulate the softmax denominator
      replicated 64x (fuses the old separate rowsum matmuls into AV)
  - epilogue per head: DVE reciprocal of the denominator rows, 64-partition
    DMA shift to align them with the AV rows, DVE multiply -> attnT fp16
  - per q-block O-projection PARTIAL (contract only this core's 256
    e-columns of Wo) followed by a 4-rank ReduceScatter(add): rank r of the
    group receives the fully-summed 128-token slice qb*512 + r*128 .. +128.
    All but the last ReduceScatter overlap with the next q-block's compute.
Host side does pure layout marshalling + fp16 casts (no arithmetic).
"""

import sys

sys.path.insert(0, "/opt/trn_rl_repo")

import numpy as np
import concourse.bass as bass
import concourse.mybir as mybir
from concourse.tile import TileContext
from concourse import bass_utils

F32 = mybir.dt.float32
F16 = mybir.dt.float16
I32 = mybir.dt.int32
AF = mybir.ActivationFunctionType
ALU = mybir.AluOpType

P = 128
E = 1024
HPC = 4  # heads per core
EC = HPC * 64  # e_out columns per core (256)
GROUPS = [[0, 1, 2, 3], [4, 5, 6, 7]]

# walrus limits sync-wait commands per instruction (fp32-class matmuls: 1).
# Split excess waits onto NoOps inserted just before, same engine.
_wait_counter = [0]


def _fix_bir_waits(raw: bytes) -> bytes:
    import orjson

    m = orjson.loads(raw)
    for fn in m["functions"]:
        for blk in fn["blocks"]:
            out = []
            changed = False
            for inst in blk["instructions"]:
                si = inst.get("sync_info") or {}
                waits = si.get("on_wait") or []
                if len(waits) > 1:
                    for w in waits[:-1]:
                        _wait_counter[0] += 1
                        out.append(
                            {
                                "engine": inst["engine"],
                                "ins": [],
                                "name": f"I-waitfix-{_wait_counter[0]}",
                                "opcode": "NoOp",
                                "outs": [],
                                "sync_info": {"on_update": [], "on_wait": [w]},
                            }
                        )
                    si["on_wait"] = waits[-1:]
                    inst["sync_info"] = si
                    changed = True
                out.append(inst)
            if changed:
                blk["instructions"] = out
    return orjson.dumps(m)


def build(S: int = 2048) -> bass.Bass:
    KC = S // 128  # k-chunks
    QBW = 512  # q-block width
    NQB = S // QBW
    OTW = QBW // P  # 128-token output slices per q-block

    nc = bass.Bass()

    xqT = nc.declare_dram_parameter("xqT", [E, S], F16, isOutput=False)
    xkT = nc.declare_dram_parameter("xkT", [E, S], F16, isOutput=False)
    xvT = nc.declare_dram_parameter("xvT", [E, S], F16, isOutput=False)
    maskT = nc.declare_dram_parameter("maskT", [S, S], F16, isOutput=False)
    WqT = nc.declare_dram_parameter("WqT", [E, EC], F16, isOutput=False)
    WkT = nc.declare_dram_parameter("WkT", [E, EC], F16, isOutput=False)
    WvT = nc.declare_dram_parameter("WvT", [E, EC], F16, isOutput=False)
    WoTs = nc.declare_dram_parameter("WoTs", [EC, E], F16, isOutput=False)
    bq = nc.declare_dram_parameter("bq", [EC], F32, isOutput=False)
    bk = nc.declare_dram_parameter("bk", [EC], F32, isOutput=False)
    bv_b = nc.declare_dram_parameter("bv_b", [P, EC], F16, isOutput=False)
    bo4_b = nc.declare_dram_parameter("bo4_b", [P, E], F16, isOutput=False)
    out = nc.declare_dram_parameter("out", [NQB * P, E], F32, isOutput=True)

    with TileContext(nc) as tc:
        with (
            tc.tile_pool(name="persist", bufs=1) as pp,
            tc.tile_pool(name="dramp", bufs=1, space="DRAM") as dramp,
        ):
            o_part = dramp.tile([NQB, QBW, E], F16)  # partial O-proj, pre-RS
            o_rs = dramp.tile([NQB, P, E], F16)  # ReduceScatter results

            qT_sb = pp.tile([P, 2, S], F16)  # [:, m, :] = q.T rows 128m..
            kT_sb = pp.tile([P, 2, S], F16)
            # AV stationary: [:, t, hh, 0:64] = v rows 128t.. of head hh;
            # [:, t, hh, 64:128] = 1.0 (fused denominator columns)
            v_st = pp.tile([P, KC, 4, P], F16)
            bq_sb = pp.tile([P, 2], F32)
            bk_sb = pp.tile([P, 2], F32)
            nc.sync.dma_start(bq_sb[:], bq.rearrange("(m p) -> p m", p=P))
            nc.sync.dma_start(bk_sb[:], bk.rearrange("(m p) -> p m", p=P))
            bv_sb = pp.tile([P, EC], F16)
            nc.gpsimd.dma_start(bv_sb[:], bv_b[:])
            bo4_sb = pp.tile([P, E], F16)
            nc.sync.dma_start(bo4_sb[:], bo4_b[:])

            wq_sb = pp.tile([P, 8, EC], F16)
            wk_sb = pp.tile([P, 8, EC], F16)
            wv_sb = pp.tile([P, 8, EC], F16)
            wo_sb = pp.tile([P, 2, E], F16)
            nc.gpsimd.dma_start(wq_sb[:], WqT.rearrange("(kt p) m -> p kt m", p=P))
            nc.gpsimd.dma_start(wk_sb[:], WkT.rearrange("(kt p) m -> p kt m", p=P))
            nc.gpsimd.dma_start(wv_sb[:], WvT.rearrange("(kt p) m -> p kt m", p=P))
            nc.gpsimd.dma_start(wo_sb[:], WoTs.rearrange("(kt p) n -> p kt n", p=P))

            # ---------------- Phase A: QKV projections ----------------
            with (
                tc.tile_pool(name="xpool", bufs=6) as xp,
                tc.tile_pool(name="psA", bufs=8, space="PSUM") as psA,
            ):
                nc.vector.memset(v_st[:, :, :, 64:128], 1.0)
                for which in range(3):
                    xT, w_sb = [(xqT, wq_sb), (xkT, wk_sb), (xvT, wv_sb)][which]
                    nps = (2 * S) // 512 if which < 2 else KC // 2
                    pst = [
                        psA.tile([P, 512], F32, name=f"psA_{which}_{i}", tag="psA")
                        for i in range(nps)
                    ]
                    for kt in range(8):
                        x_t = xp.tile([P, S], F16, name=f"x_{which}_{kt}", tag="x")
                        # one DMA queue per x tensor: q->sync, k->scalar,
                        # v->gpsimd (three parallel streams in phase A)
                        eng = [nc.sync, nc.scalar, nc.gpsimd][which]
                        eng.dma_start(x_t[:], xT[kt * P : (kt + 1) * P, :])
                        if which < 2:
                            # q.T / k.T: out [256, S]; lhsT = W tile, rhs = x.T
                            for m in range(2):
                                lhsT = w_sb[:, kt, m * P : (m + 1) * P]
                                for n in range(S // 512):
                                    nc.tensor.matmul(
                                        pst[m * (S // 512) + n][:],
                                        lhsT,
                                        x_t[:, n * 512 : (n + 1) * 512],
                                        start=(kt == 0),
                                        stop=(kt == 7),
                                    )
                        else:
                            # v: out [S, 256]; lhsT = x.T tile, rhs = W k-tile.
                            # Two token-chunks share one PSUM bank: the
                            # has_written group opens on the even chunk and
                            # closes on the odd one (2KB zero-region rule).
                            for t in range(KC):
                                nc.tensor.matmul(
                                    pst[t // 2][:, (t % 2) * EC : (t % 2 + 1) * EC],
                                    x_t[:, t * P : (t + 1) * P],
                                    w_sb[:, kt, :],
                                    start=(kt == 0 and t % 2 == 0),
                                    stop=(kt == 7 and t % 2 == 1),
                                )
                    if which == 0:
                        for m in range(2):
                            for n in range(S // 512):
                                # (q + bq) / 8, bias before scale
                                nc.vector.tensor_scalar(
                                    qT_sb[:, m, n * 512 : (n + 1) * 512],
                                    pst[m * (S // 512) + n][:],
                                    bq_sb[:, m : m + 1],
                                    0.125,
                                    ALU.add,
                                    ALU.mult,
                                )
                    elif which == 1:
                        for m in range(2):
                            for n in range(S // 512):
                                nc.vector.tensor_scalar(
                                    kT_sb[:, m, n * 512 : (n + 1) * 512],
                                    pst[m * (S // 512) + n][:],
                                    1.0,
                                    bk_sb[:, m : m + 1],
                                    ALU.mult,
                                    ALU.add,
                                )
                    else:
                        for t in range(KC):
                            nc.vector.tensor_tensor(
                                v_st[:, t, :, 0:64],
                                pst[t // 2][
                                    :, (t % 2) * EC : (t % 2 + 1) * EC
                                ].rearrange("p (h d) -> p h d", h=4),
                                bv_sb[:].rearrange("p (h d) -> p h d", h=4),
                                ALU.add,
                            )

            # ---------- Phase B: attention + partial O-proj + ReduceScatter ----
            with (
                tc.tile_pool(name="maskpool", bufs=1) as mp,
                tc.tile_pool(name="ppool", bufs=7) as ppl,
                tc.tile_pool(name="epool", bufs=2) as ep,
                tc.tile_pool(name="atpool", bufs=2) as atp,
                tc.tile_pool(name="opool", bufs=2) as op,
                tc.tile_pool(name="bps", bufs=2, space="PSUM") as bps,
                tc.tile_pool(name="avps", bufs=4, space="PSUM") as avps,
            ):
                maskbf = mp.tile([P, KC, S], F16)
                for t in range(KC):
                    nc.gpsimd.dma_start(
                        maskbf[:, t, :], maskT[t * P : (t + 1) * P, :]
                    )

                LAG = 4  # QK runs this many k-chunks ahead of AV on the PE
                av_tiles = {}
                pws = {}
                attnTs = {}

                def emit_chunk(qb, pair, kc_from, kc_to):
                    qsl = slice(qb * QBW, (qb + 1) * QBW)
                    av_t = av_tiles.get((qb, pair))  # absent during warmup
                    pw = pws[(qb, pair)]
                    for kc in range(kc_from, kc_to):
                        if kc < KC:
                            ksl = slice(kc * P, (kc + 1) * P)
                            s_t = bps.tile(
                                [P, 2, QBW], F32, name=f"s_{qb}_{pair}_{kc}", tag="b"
                            )
                            for h in range(2):
                                prt = slice(h * 64, (h + 1) * 64)
                                nc.tensor.matmul(
                                    s_t[:, h, :],
                                    kT_sb[prt, pair, ksl],
                                    qT_sb[prt, pair, qsl],
                                    start=True,
                                    stop=True,
                                )
                            p_t = ppl.tile([P, 2 * QBW], F16, name="p_t", tag="p")
                            pw[kc] = p_t
                            nc.scalar.activation(
                                p_t[:].rearrange("p (h n) -> p h n", h=2),
                                s_t[:],
                                AF.Exp,
                            )
                            nc.vector.tensor_tensor(
                                p_t[:].rearrange("p (h n) -> p h n", h=2),
                                p_t[:].rearrange("p (h n) -> p h n", h=2),
                                maskbf[:, kc, qsl][:, None, :].to_broadcast(
                                    (P, 2, QBW)
                                ),
                                ALU.mult,
                            )
                        if kc >= LAG:
                            ka = kc - LAG
                            for h in range(2):
                                # stationary [v_h | ones64]: rows 0-63 = AV,
                                # rows 64-127 = denominator replicated x64
                                nc.tensor.matmul(
                                    av_t[h][:],
                                    v_st[:, ka, pair * 2 + h, :],
                                    pw[ka][:, h * QBW : (h + 1) * QBW],
                                    start=(ka == 0),
                                    stop=(ka == KC - 1),
                                )

                def emit_epilogue(qb, pair):
                    # divide by denominator -> attnT[:, pair, :].  DVE operand
                    # partition windows are independent, so the multiply reads
                    # the AV rows (0-63) and the reciprocal rows (64-127)
                    # directly -- no partition-shift DMA needed.
                    av_t = av_tiles[(qb, pair)]
                    for h in range(2):
                        rb = ep.tile([P, QBW], F32, name="rb", tag="rb")
                        nc.vector.reciprocal(rb[64:128, :], av_t[h][64:128, :])
                        nc.vector.tensor_tensor(
                            attnTs[qb][h * 64 : (h + 1) * 64, pair, :],
                            av_t[h][0:64, :],
                            rb[64:128, :],
                            ALU.mult,
                        )

                def emit_oproj(qb):
                    # partial O-projection (own 256-column e-slice) + RS
                    attnT = attnTs[qb]
                    for qt in range(OTW):
                        for n in range(2):
                            o_ps = avps.tile(
                                [P, QBW], F32, name=f"o_{qb}_{qt}_{n}", tag="av"
                            )
                            for kt in range(2):
                                nc.tensor.matmul(
                                    o_ps[:],
                                    attnT[:, kt, qt * P : (qt + 1) * P],
                                    wo_sb[:, kt, n * 512 : (n + 1) * 512],
                                    start=(kt == 0),
                                    stop=(kt == 1),
                                )
                            # drain with bo/4 folded: the 4-rank RS adds bo
                            o_f = op.tile([P, QBW], F16, name="o_f", tag="of")
                            nc.vector.tensor_tensor(
                                o_f[:],
                                o_ps[:],
                                bo4_sb[:, n * 512 : (n + 1) * 512],
                                ALU.add,
                            )
                            nc.sync.dma_start(
                                o_part[qb, qt * P : (qt + 1) * P, n * 512 : (n + 1) * 512],
                                o_f[:],
                            )
                    nc.gpsimd.collective_compute(
                        "ReduceScatter",
                        ALU.add,
                        ins=[o_part[qb]],
                        outs=[o_rs[qb]],
                        replica_groups=GROUPS,
                    )
                    # final output: one casting DMA on the gpsimd queue
                    nc.gpsimd.dma_start(out[qb * P : (qb + 1) * P, :], o_rs[qb])

                pending_oproj = None
                prev_pair = None
                for qb in range(NQB):
                    attnTs[qb] = atp.tile([P, 2, QBW], F16, name=f"at_{qb}", tag="at")
                    for pair in range(2):
                        pws[(qb, pair)] = [None] * KC
                        if pair == 0 and prev_pair is not None:
                            # qb boundary: epilogue first so the O-proj flush
                            # below unblocks as early as possible
                            emit_epilogue(*prev_pair)
                            prev_pair = None
                        # warmup (QK/exp/mask only)
                        emit_chunk(qb, pair, 0, LAG)
                        if prev_pair is not None:
                            # pair boundary: epilogue after the warmup masks
                            # (nothing needs attnT soon; masks gate the AVs)
                            emit_epilogue(*prev_pair)
                            prev_pair = None
                        if pending_oproj is not None:
                            emit_oproj(pending_oproj)
                            pending_oproj = None
                        # allocate AV accumulators after the O-proj tiles so
                        # the PSUM ring slots they take are already past them
                        av_tiles[(qb, pair)] = [
                            avps.tile(
                                [P, QBW], F32, name=f"av_{qb}_{pair}_{h}", tag="av"
                            )
                            for h in range(2)
                        ]
                        emit_chunk(qb, pair, LAG, KC + LAG)
                        prev_pair = (qb, pair)
                    pending_oproj = qb
                emit_epilogue(*prev_pair)
                emit_oproj(NQB - 1)

    fixed = _fix_bir_waits(nc.to_json_bytes())
    nc.to_json_bytes = lambda: fixed
    return nc


_NC_CACHE: dict = {}


def _get_nc(S: int) -> bass.Bass:
    if S not in _NC_CACHE:
        _NC_CACHE[S] = build(S)
    return _NC_CACHE[S]


def kernel(
    query,
    key,
    value,
    mask,
    Wq,
    bq,
    Wk,
    bk,
    Wv,
    bv,
    Wo,
    bo,
    _trace: bool = False,
    _trace_dir: str | None = None,
):
    query = np.asarray(query, np.float32)
    key = np.asarray(key, np.float32)
    value = np.asarray(value, np.float32)
    mask = np.asarray(mask, np.int32)
    Wq = np.asarray(Wq, np.float32)
    Wk = np.asarray(Wk, np.float32)
    Wv = np.asarray(Wv, np.float32)
    Wo = np.asarray(Wo, np.float32)
    bq = np.asarray(bq, np.float32)
    bk = np.asarray(bk, np.float32)
    bv = np.asarray(bv, np.float32)
    bo = np.asarray(bo, np.float32)

    B, S, E_ = query.shape
    assert (B, E_) == (2, 1024), (B, E_)
    nc = _get_nc(S)

    # host-side layout marshalling + fp16 casts (no arithmetic)
    xT = {}
    for g in range(2):
        xT[("q", g)] = np.ascontiguousarray(query[g].T.astype(np.float16))
        xT[("k", g)] = np.ascontiguousarray(key[g].T.astype(np.float16))
        xT[("v", g)] = np.ascontiguousarray(value[g].T.astype(np.float16))
    maskTt = [np.ascontiguousarray(mask[g].T.astype(np.float16)) for g in range(2)]
    WoT_h = np.ascontiguousarray(Wo.T.astype(np.float16))  # [e_in, e_out]
    bo4_rep = np.ascontiguousarray(
        np.broadcast_to((bo / 4.0).astype(np.float16), (128, 1024))
    )

    in_maps = []
    for c in range(8):
        g, r = divmod(c, 4)
        hs = slice(r * EC, (r + 1) * EC)
        in_maps.append(
            {
                "xqT": xT[("q", g)],
                "xkT": xT[("k", g)],
                "xvT": xT[("v", g)],
                "maskT": maskTt[g],
                "WqT": np.ascontiguousarray(Wq[hs, :].T.astype(np.float16)),
                "WkT": np.ascontiguousarray(Wk[hs, :].T.astype(np.float16)),
                "WvT": np.ascontiguousarray(Wv[hs, :].T.astype(np.float16)),
                "WoTs": np.ascontiguousarray(WoT_h[hs, :]),
                "bq": np.ascontiguousarray(bq[hs]),
                "bk": np.ascontiguousarray(bk[hs]),
                "bv_b": np.ascontiguousarray(
                    np.broadcast_to(bv[hs].astype(np.float16), (128, EC))
                ),
                "bo4_b": bo4_rep,
            }
        )

    kw = {}
    if _trace:
        kw = dict(trace=True, tmpdir=_trace_dir)
    res = bass_utils.run_bass_kernel_spmd(nc, in_maps, list(range(8)), **kw)

    # core c=4g+r, result row qb*128+j  ->  out_full[g, qb*512 + r*128 + j]
    out_full = np.empty((B, S, E_), np.float32)
    for c in range(8):
        g, r = divmod(c, 4)
        blk = res.results[c]["out"].reshape(4, P, E_)
        out_full[g].reshape(4, 4, P, E_)[:, r, :, :] = blk
    if _trace:
        kernel._last_exec_time_ns = res.exec_time_ns
        kernel._last_trace = res.instructions_and_trace
    return out_full
